# revision 39
# baseline (speedup 1.0000x reference)
"""Trainium2 Bass kernel for the attention-scoring module:

    out[b, s] = softmax_s( (enc[b] @ W.T + bias) @ h[b] )

Math: the bias term contributes a constant per (b, :) row, which cancels in
the softmax, and the two contractions reassociate:

    energies[b, s] = enc[b, s, :] . v[b]   with   v[b] = h[b] @ W

Sharding: data-parallel over batch — one batch per NeuronCore (B == 8 cores).

Per-core schedule (DMA engines are the serializing resource at ~360 GB/s):
  - W (1 MB) first, then enc[b] (16 MB) streamed as 64 uniform 128-row
    chunks.  A 128-row chunk transfers in ~728 ns while its fused
    multiply+row-sum DVE instruction takes ~640 ns, so the DVE tracks
    arrivals with no terminal backlog and the last energy column is ready
    ~1.5 us after the last HBM byte.
  - softmax shift comes from the first 32 columns mid-stream; exp/transpose
    of cols 0..60 also run mid-stream.  The tail after the final chunk is
    exp of 4 columns + sum + reciprocal + scale.
  - the output store is a SWDGE kv_writeback prepared mid-stream; the final
    trigger_dma fires pre-generated descriptors, skipping the HWDGE launch
    latency on the critical path.
"""

from contextlib import ExitStack

import numpy as np

import concourse.tile as tile
from concourse import bacc, mybir
from concourse.bass_utils import run_bass_kernel_spmd
from concourse.masks import make_identity

B, S, H = 8, 8192, 512
N_CORES = 8
P = 128
N_COLS = S // P  # 64 energy columns, E[p, t] = energy(s = t*128 + p)
F32 = mybir.dt.float32
I32 = mybir.dt.int32
ALU = mybir.AluOpType
ACTF = mybir.ActivationFunctionType
AXX = mybir.AxisListType.X

CHUNK_BUFS = 16
EC = 32   # softmax shift comes from the first 32 columns, mid-stream
MC = 60   # second exp/transpose stage covers cols EC..MC


def _build_kernel():
    nc = bacc.Bacc("TRN2", target_bir_lowering=False, debug=False)
    enc = nc.dram_tensor("enc", [S, H], F32, kind="ExternalInput")
    hvec = nc.dram_tensor("hvec", [1, H], F32, kind="ExternalInput")
    Wmat = nc.dram_tensor("W", [H, H], F32, kind="ExternalInput")
    out = nc.dram_tensor("out", [S], F32, kind="ExternalOutput")

    with ExitStack() as ctx:
        tc = ctx.enter_context(tile.TileContext(nc))
        consts = ctx.enter_context(tc.tile_pool(name="consts", bufs=1))
        small = ctx.enter_context(tc.tile_pool(name="small", bufs=1))
        chunks = ctx.enter_context(tc.tile_pool(name="chunks", bufs=CHUNK_BUFS))
        scratch = ctx.enter_context(tc.tile_pool(name="scratch", bufs=2))
        psum = ctx.enter_context(tc.tile_pool(name="psum", bufs=1, space="PSUM"))
        psum1 = ctx.enter_context(tc.tile_pool(name="psum1", bufs=1, space="PSUM"))

        # Constants.
        identity = consts.tile([P, P], F32)
        make_identity(nc, identity[:])
        one11 = consts.tile([1, 1], F32)
        nc.gpsimd.memset(one11[:], 1.0)
        ones_row = consts.tile([1, P], F32)
        nc.gpsimd.memset(ones_row[:], 1.0)
        neg_ones_row = consts.tile([1, P], F32)
        nc.gpsimd.memset(neg_ones_row[:], -1.0)
        ones_64 = consts.tile([P, N_COLS], F32)
        nc.gpsimd.memset(ones_64[:], 1.0)

        # Output staging: probsT[t, p] = prob(s = t*128 + p).
        probsT_sb = small.tile([EC, P], F32)  # unscaled exp, rows 0..EC
        # `final` is a raw SBUF tensor, not a pool tile: the pool-close Drain
        # instructions then don't wait for the output DMA that reads it, so
        # the epilogue overlaps the store instead of trailing it.
        final_t = ctx.enter_context(nc.sbuf_tensor("final", [N_COLS, P], F32))
        final = final_t.ap()

        # ---- queue the input DMAs: hvec, W, then 64 uniform enc chunks ----
        # W halves first (big transfers keep the DMA bus busy while the SP
        # sequencer's issue pipeline ramps), then hvec, then the enc chunks.
        W_h = []
        for g in range(2):
            wh = small.tile([P, 2, H], F32, tag=f"wh{g}")
            W_h.append(wh)
            nc.sync.dma_start(
                wh[:],
                Wmat.ap()[g * 2 * P : (g + 1) * 2 * P, :].rearrange(
                    "(c p) h -> p c h", c=2, p=P
                ),
            )
        W_c = [W_h[0][:, 0, :], W_h[0][:, 1, :], W_h[1][:, 0, :], W_h[1][:, 1, :]]
        hrow = small.tile([1, H], F32)
        nc.sync.dma_start(hrow[:], hvec.ap())

        # Prepared output store: kv_writeback descriptors are generated here
        # (mid-stream, off the critical path); the trailing trigger_dma fires
        # them.  out[0, t, p, idx=0] = final[t, p]  ->  out[t*128 + p].
        # Trigger the ACT exp table load at t=0 instead of in the tail.
        dummy_act = small.tile([1, 1], F32)
        nc.scalar.activation(dummy_act[:], one11[:], ACTF.Exp, bias=0.0, scale=1.0)

        # ---- v = h @ W, broadcast to all 128 partitions ----
        # PE p-state ramps LOW -> MID -> full over ~3us of continuous work;
        # a train of dummy transposes warms it so the fp32 v matmuls (4
        # cycles/row) run at full clock when W arrives.
        warm_tiles = []
        for i in range(24):
            wt = psum1.tile([P, P], F32, tag=f"htb{i % 2}")
            warm_tiles.append(wt)
            nc.tensor.transpose(wt[:], identity[:], identity[:])
        # Stage 1 fuses transpose+broadcast: hTb_c[m, n] = h[c*128+m] for all
        # n (a single matmul: hrow-chunk stationary x ones_row moving).
        # Stage 2 contracts: v_bc[m, n] = sum_c sum_p hTb_c[p, m] * W_c[p, n]
        # = sum_k h[k] W[k, n], identical on every output partition m.
        hT_sb = []
        for c in range(4):
            hT_ps = psum1.tile([P, P], F32, tag=f"htb{c % 2}")
            nc.tensor.matmul(
                hT_ps[:],
                hrow[:1, c * P : (c + 1) * P],
                ones_row[:],
                start=True,
                stop=True,
            )
            ht = small.tile([P, P], F32, tag=f"ht{c}")
            hT_sb.append(ht)
            nc.scalar.copy(ht[:], hT_ps[:])
        v_bc_ps = psum1.tile([P, H], F32, tag="vbc")
        for c in range(4):
            nc.tensor.matmul(
                v_bc_ps[:],
                hT_sb[c][:],
                W_c[c],
                start=(c == 0),
                stop=(c == 3),
            )
        v_sb = small.tile([P, H], F32)
        nc.scalar.copy(v_sb[:], v_bc_ps[:])

        # ---- main loop: stream enc, fused multiply+reduce on DVE ----
        E = small.tile([P, N_COLS], F32)
        P_exp = small.tile([P, N_COLS + 1], F32)  # col 64 holds rs12
        rs1 = small.tile([P, 1], F32)
        negM_sb = small.tile([P, 1], F32)
        negM_ps = psum.tile([P, 1], F32, tag="colp")
        probsT_ps = psum.tile([EC, P], F32, tag="outp")
        probsT23_ps = psum.tile([N_COLS - EC, P], F32, tag="outp2")

        def emit_shift_chain():
            # Softmax shift from the first EC columns, computed mid-stream.
            # Any shift within ~80 of the true max keeps exp() finite, and
            # the shift cancels exactly in the final normalization.
            m_col = small.tile([P, 1], F32)
            nc.vector.tensor_reduce(m_col[:], E[:, :EC], axis=AXX, op=ALU.max)
            gmax = small.tile([1, 1], F32)
            nc.gpsimd.tensor_reduce(
                gmax[:], m_col[:], axis=mybir.AxisListType.C, op=ALU.max
            )
            # broadcast -shift to all partitions via matmul with -1s
            nc.tensor.matmul(
                negM_ps[:], neg_ones_row[:], gmax[:], start=True, stop=True
            )
            nc.scalar.copy(negM_sb[:], negM_ps[:])
            # exp + row-sum + transpose of the early columns, off critical path
            nc.scalar.activation(
                P_exp[:, :EC],
                E[:, :EC],
                ACTF.Exp,
                bias=negM_sb[:],
                scale=1.0,
                accum_out=rs1[:],
            )
            nc.tensor.transpose(probsT_ps[:], P_exp[:, :EC], identity[:])
            nc.scalar.copy(probsT_sb[:], probsT_ps[:])

        def emit_mid_chain():
            # exp of cols EC..MC; rs12 = rs1 + rs2 lands in P_exp[:, 64] so
            # the tail reduce covers it for free.  The transpose of cols
            # EC..64 happens once, in the tail.
            rs2 = small.tile([P, 1], F32)
            nc.scalar.activation(
                P_exp[:, EC:MC],
                E[:, EC:MC],
                ACTF.Exp,
                bias=negM_sb[:],
                scale=1.0,
                accum_out=rs2[:],
            )
            nc.vector.tensor_add(P_exp[:, N_COLS : N_COLS + 1], rs1[:], rs2[:])

        for t in range(N_COLS):
            ch = chunks.tile([P, H], F32, tag="chunk")
            nc.sync.dma_start(ch[:], enc.ap()[t * P : (t + 1) * P, :])
            prod = scratch.tile([P, H], F32, tag="prod")
            # fused multiply + free-dim sum in one DVE instruction
            nc.vector.scalar_tensor_tensor(
                out=prod[:],
                in0=ch[:],
                scalar=1.0,
                in1=v_sb[:],
                op0=ALU.bypass,
                op1=ALU.mult,
                accum_out=E[:, t : t + 1],
            )
            if t + 1 == EC:
                emit_shift_chain()
            if t + 1 == MC:
                emit_mid_chain()

        # ---- softmax tail: only cols MC..64 remain ----
        nc.scalar.activation(
            P_exp[:, MC:N_COLS],
            E[:, MC:N_COLS],
            ACTF.Exp,
            bias=negM_sb[:],
            scale=1.0,
        )
        # transpose of cols EC..64 (PE) runs parallel to the sum chain (DVE)
        nc.tensor.transpose(probsT23_ps[:], P_exp[:, EC:N_COLS], identity[:])
        # total row-sum: tail exps + rs12 in one reduce
        rs_tot = small.tile([P, 1], F32)
        nc.vector.tensor_reduce(
            rs_tot[:], P_exp[:, MC : N_COLS + 1], axis=AXX, op=ALU.add
        )
        # S broadcast to 64 partitions: ones^T @ rs_tot
        Sb_ps = psum.tile([N_COLS, 1], F32, tag="sb")
        nc.tensor.matmul(Sb_ps[:], ones_64[:], rs_tot[:], start=True, stop=True)
        SinvB = small.tile([N_COLS, 1], F32)
        nc.vector.reciprocal(SinvB[:], Sb_ps[:])
        # scale (SinvB entries are identical, so base-0 slices are valid)
        nc.vector.tensor_scalar_mul(
            final[EC:, :], probsT23_ps[:], SinvB[: N_COLS - EC, :]
        )
        nc.vector.tensor_scalar_mul(final[:EC, :], probsT_sb[:], SinvB[:EC, :])
        nc.sync.dma_start(out.ap().rearrange("(t p) -> t p", p=P), final)

    nc.compile()
    return nc


_NC_CACHE = {}


def kernel(hidden, encoder_outputs, W, b):
    """Full (unsharded) inputs in, full output out; 8-core SPMD inside."""
    if "nc" not in _NC_CACHE:
        _NC_CACHE["nc"] = _build_kernel()
    nc = _NC_CACHE["nc"]

    hidden = np.asarray(hidden)
    enc = np.ascontiguousarray(np.asarray(encoder_outputs, dtype=np.float32))
    Wm = np.ascontiguousarray(np.asarray(W, dtype=np.float32))
    in_maps = [
        {
            "enc": enc[c],
            "hvec": np.ascontiguousarray(hidden[0, c][None, :].astype(np.float32)),
            "W": Wm,
        }
        for c in range(N_CORES)
    ]
    res = run_bass_kernel_spmd(nc, in_maps, core_ids=list(range(N_CORES)))
    return np.stack([res.results[c]["out"] for c in range(N_CORES)], axis=0).astype(
        np.float32
    )
